# revision 51
# baseline (speedup 1.0000x reference)
"""Multi-head attention TRN2 kernel (8 NeuronCores).

Problem: B=4, S=2048, D_IN=768, H=12, D_HEAD=64.
  q/k/v = einsum('hkd,bsk->bhsd', w{q,k,v}, x)
  out   = einsum('ij,bsj->bsi', wc, softmax(q k^T / 8) v  concat-heads)

Sharding: 8 cores = (batch b in 0..3) x (head-half in 0..1), 6 heads per core.
Each core computes, per head: Q^T,K^T [64,2048] fp16 projections, a
max-finding scores pass in [l,n] layout (per-block DVE max-reduces
straight from PSUM; DVE is the only engine that can free-axis-reduce
and may read at most one PSUM operand per instruction), a second scores
pass in [n,l] layout with the per-row max folded in via an augmented
contraction row, exp on the scalar engine (PSUM fp32 -> SBUF fp16),
and attn@V with an appended
ones-column producing unnormalized z^T plus the softmax normalizer Z.
The host divides by Z, concatenates heads and applies the (tiny) output
projection in fp32 BLAS.

All matmuls run in fp16 (fp32 PSUM accumulation).
"""

import numpy as np

B, S, D_IN, H, D_HEAD = 4, 2048, 768, 12, 64
HL = H // 2          # heads per core
KC = D_IN // 128     # k chunks
N_CORES = 8

_CACHE = {}
CFG = {'shared_pool': False, 'zsb_engine': 'act', 'qasm_engine': 'dma', 'p1_prio': 30, 'act_blocks': frozenset(), 'combine_engine': 'dve', 'depth': 3, 'pre_skip': 0, 'v_first': True, 'sbp_bufs': 3, 'psav_bufs': 3, 'k_copy_dve': False, 'split_x0': False, 'wide_p1': False, 'rate_slack': 8, 'rot_p1': False, 'fused_in': True, 'fused_step': 1}


def build_bass():
    import concourse.bass as bass
    import concourse.bacc as bacc
    import concourse.mybir as mybir
    import concourse.tile as tile
    from contextlib import ExitStack

    f16 = mybir.dt.float16
    f32 = mybir.dt.float32
    AX = mybir.AxisListType
    ALU = mybir.AluOpType
    ACT_EXP = mybir.ActivationFunctionType.Exp

    nc = bacc.Bacc()
    XW = S + 3 * HL * 64
    if CFG.get('fused_in'):
        xw_d = nc.declare_dram_parameter("xw", [D_IN, XW], f16, isOutput=False)
    else:
        xT_d = nc.declare_dram_parameter("xT", [D_IN, S], f16, isOutput=False)
        wq_d = nc.declare_dram_parameter("wq", [D_IN, HL * 64], f16, isOutput=False)
        wk_d = nc.declare_dram_parameter("wk", [D_IN, HL * 64], f16, isOutput=False)
        wv_d = nc.declare_dram_parameter("wv", [D_IN, HL * 64], f16, isOutput=False)
    zu_d = nc.declare_dram_parameter("zu", [HL, 65, S], f32, isOutput=True)

    with tile.TileContext(nc) as tc, ExitStack() as ctx:
        consts = ctx.enter_context(tc.tile_pool(name="consts", bufs=1))

        # ---- persistent SBUF tensors ----
        if CFG.get('fused_in'):
            # all inputs in one tensor -> 6 chunk DMAs instead of 24,
            # collapsing the serialized HWDGE descriptor-gen chain
            xw_sb = consts.tile([128, KC, XW], f16)
            xT_sb = xw_sb[:, :, 0:S]
            wq_sb = xw_sb[:, :, S:S + HL * 64]
            wk_sb = xw_sb[:, :, S + HL * 64:S + 2 * HL * 64]
            wv_sb = xw_sb[:, :, S + 2 * HL * 64:S + 3 * HL * 64]
            step = CFG.get('fused_step', 1)
            if CFG.get('multi_q'):
                qs = [nc.sync, nc.scalar]
                for c0 in range(0, KC, step):
                    qs[(c0 // step) % len(qs)].dma_start(
                        out=xw_sb[:, c0:c0 + step, :],
                        in_=xw_d[c0 * 128:(c0 + step) * 128, :].rearrange(
                            "(c p) w -> p c w", c=step))
            else:
                for c0 in range(0, KC, step):
                    nc.sync.dma_start(
                        out=xw_sb[:, c0:c0 + step, :],
                        in_=xw_d[c0 * 128:(c0 + step) * 128, :].rearrange(
                            "(c p) w -> p c w", c=step))
        else:
            xT_sb = consts.tile([128, KC, S], f16)
            wq_sb = consts.tile([128, KC, HL * 64], f16)
            wk_sb = consts.tile([128, KC, HL * 64], f16)
            wv_sb = consts.tile([128, KC, HL * 64], f16)
        if (not CFG.get('fused_in')) and CFG.get('split_x0') == 'lead':
            # lead slice: wq c0 + first 512 cols of xT c0 land first so the
            # first projection matmul starts ~2us earlier
            nc.sync.dma_start(out=wq_sb[:, 0, :], in_=wq_d[0:128, :])
            nc.sync.dma_start(out=xT_sb[:, 0, 0:512], in_=xT_d[0:128, 0:512])
            nc.sync.dma_start(out=wk_sb[:, 0, :], in_=wk_d[0:128, :])
            nc.sync.dma_start(out=xT_sb[:, 0, 512:2048], in_=xT_d[0:128, 512:2048])
            for c in range(1, KC):
                nc.sync.dma_start(out=xT_sb[:, c, :], in_=xT_d[c * 128:(c + 1) * 128, :])
                nc.sync.dma_start(out=wq_sb[:, c, :], in_=wq_d[c * 128:(c + 1) * 128, :])
                nc.sync.dma_start(out=wk_sb[:, c, :], in_=wk_d[c * 128:(c + 1) * 128, :])
        elif (not CFG.get('fused_in')) and CFG.get('split_x0'):
            nc.sync.dma_start(out=wq_sb[:, 0, :], in_=wq_d[0:128, :])
            for sc in range(4):
                ssl = slice(sc * 512, (sc + 1) * 512)
                nc.sync.dma_start(out=xT_sb[:, 0, ssl], in_=xT_d[0:128, ssl])
            nc.sync.dma_start(out=wk_sb[:, 0, :], in_=wk_d[0:128, :])
            for c in range(1, KC):
                nc.sync.dma_start(out=xT_sb[:, c, :], in_=xT_d[c * 128:(c + 1) * 128, :])
                nc.sync.dma_start(out=wq_sb[:, c, :], in_=wq_d[c * 128:(c + 1) * 128, :])
                nc.sync.dma_start(out=wk_sb[:, c, :], in_=wk_d[c * 128:(c + 1) * 128, :])
        elif not CFG.get('fused_in'):
            for c in range(KC):
                nc.sync.dma_start(out=xT_sb[:, c, :], in_=xT_d[c * 128:(c + 1) * 128, :])
                nc.sync.dma_start(out=wq_sb[:, c, :], in_=wq_d[c * 128:(c + 1) * 128, :])
                nc.sync.dma_start(out=wk_sb[:, c, :], in_=wk_d[c * 128:(c + 1) * 128, :])
        if not CFG.get('fused_in'):
            for c in range(KC):
                nc.sync.dma_start(out=wv_sb[:, c, :], in_=wv_d[c * 128:(c + 1) * 128, :])

        if CFG.get('zu_batch'):
            zout_sb = consts.tile([65, 4, 512], f32, name="zout_sb")

        # per-head Q^T [64, S];  K~^T [65, S] with ones row;  V~ [128, 16, h, 65] with ones col
        qT = [consts.tile([64, S], f16, name=f"qT{h}", tag=f"qT{h}") for h in range(HL)]
        kT = [consts.tile([65, S], f16, name=f"kT{h}", tag=f"kT{h}") for h in range(HL)]
        v_all = consts.tile([128, 16, HL, 65], f16)
        for h in range(HL):
            nc.gpsimd.memset(kT[h][64:65, :], 1.0)
        nc.gpsimd.memset(v_all[:, :, :, 64:65], 1.0)


        # ---- attention, software-pipelined at depth D over phase A ----
        # steps = (head, l-chunk).  pass-1 (max-finding, DVE-bound) of step
        # i+1 is interleaved at per-group granularity with pass-2
        # (exp + attn@V) of step i.  Additionally the first D steps' pass-1
        # is emitted under the phase-A projection matmuls (PE-bound): pq/pk
        # are single-buffered so psA takes only 4 banks, and the 3-bank
        # pass-1 pool opens alongside it.  After psA closes, pass-2's 4
        # banks + zt open in the freed space.
        D = CFG['depth']
        from contextlib import ExitStack as _ES
        _pools = _ES()
        p1p = _pools.enter_context(tc.tile_pool(name="p1", bufs=(2 if CFG.get("wide_p1") else (1 if CFG.get('rot_p1') else 3)), space="PSUM"))
        if CFG.get('rot_p1'):
            t1r = p1p.tile([128, 1536], f32, name="t1r", tag="t1")
            rot = {'nb': 0}
        with _pools, tc.tile_pool(name="sbp", bufs=CFG.get("sbp_bufs", 3)) as sbp, \
             tc.tile_pool(name="qsp", bufs=D + 2) as qsp, \
             tc.tile_pool(name="smp", bufs=D + 1) as smp:

            def p1_chunk(h, lc, g, mcat, red):
                # pass-1 pair g (ls = g//2, n-half = g%2): two 512-n score
                # blocks, each max-reduced into a red column.  DVE can read
                # only one PSUM operand per instruction, so blocks reduce
                # independently; a few per step go via ACT copy + fp16
                # reduce to balance engine load.  After the last block of an
                # ls, the four red columns collapse (negated) into mcat[:,ls].
                ls, half = divmod(g, 2)
                l0 = lc * 512 + ls * 128
                if CFG.get('rot_p1'):
                    # manual 3-bank rotation: banks (nb, nb+1); non-wrapping
                    # pairs get one 1024-wide reduce, wrapping pairs two
                    # narrow ones.  WAR on region slices replaces pool frees.
                    nb = rot['nb']
                    b2 = (nb + 1) % 3
                    rot['nb'] = (nb + 2) % 3
                    for j, b in enumerate((nb, b2)):
                        nc.tensor.matmul(
                            t1r[:, b * 512:(b + 1) * 512], qT[h][:, l0:l0 + 128],
                            kT[h][0:64, (2 * half + j) * 512:(2 * half + j + 1) * 512],
                            start=True, stop=True)
                    sbase = 4 * ls
                    ns_used = red_slots.setdefault((h, lc, ls), [0])
                    if b2 == nb + 1:
                        nc.vector.tensor_reduce(
                            red[:, sbase + ns_used[0]:sbase + ns_used[0] + 1],
                            t1r[:, nb * 512:(nb + 2) * 512], axis=AX.X, op=ALU.max)
                        ns_used[0] += 1
                    else:
                        for b in (nb, b2):
                            nc.vector.tensor_reduce(
                                red[:, sbase + ns_used[0]:sbase + ns_used[0] + 1],
                                t1r[:, b * 512:(b + 1) * 512], axis=AX.X, op=ALU.max)
                            ns_used[0] += 1
                    if half == 1:
                        nc.vector.tensor_reduce(
                            mcat[:, ls:ls + 1], red[:, sbase:sbase + ns_used[0]],
                            axis=AX.X, op=ALU.max, negate=True)
                        del red_slots[(h, lc, ls)]
                    return
                if CFG.get('wide_p1'):
                    t1 = p1p.tile([128, 1024], f32, tag="t1", name="t1")
                    for j in range(2):
                        nc.tensor.matmul(
                            t1[:, j * 512:(j + 1) * 512], qT[h][:, l0:l0 + 128],
                            kT[h][0:64, (2 * half + j) * 512:(2 * half + j + 1) * 512],
                            start=True, stop=True)
                    nc.vector.tensor_reduce(
                        red[:, 2 * ls + half:2 * ls + half + 1],
                        t1, axis=AX.X, op=ALU.max)
                    if half == 1:
                        nc.vector.tensor_reduce(
                            mcat[:, ls:ls + 1], red[:, 2 * ls:2 * ls + 2],
                            axis=AX.X, op=ALU.max, negate=True)
                    return
                for j in range(2):
                    blk = g * 2 + j
                    tj = p1p.tile([128, 512], f32, tag="t1", name="tj")
                    nc.tensor.matmul(
                        tj, qT[h][:, l0:l0 + 128],
                        kT[h][0:64, (2 * half + j) * 512:(2 * half + j + 1) * 512],
                        start=True, stop=True)
                    if blk in CFG['act_blocks']:
                        cp = sbp.tile([128, 512], f16, tag="cp", name="cp")
                        with tc.high_priority(offset=CFG.get('cp_prio', 200)):
                            nc.scalar.copy(cp, tj)
                        nc.vector.tensor_reduce(
                            red[:, 4 * ls + 2 * half + j: 4 * ls + 2 * half + j + 1],
                            cp, axis=AX.X, op=ALU.max)
                    else:
                        nc.vector.tensor_reduce(
                            red[:, 4 * ls + 2 * half + j: 4 * ls + 2 * half + j + 1],
                            tj, axis=AX.X, op=ALU.max)
                if half == 1:
                    nc.vector.tensor_reduce(
                        mcat[:, ls:ls + 1], red[:, 4 * ls:4 * ls + 4],
                        axis=AX.X, op=ALU.max, negate=True)

            def p1_finish(h, lc, mcat, red):
                # -max cols [128, 0:4] -> XBAR DMA transpose -> mT[0:4, :]
                # holds -max for l = ls*128 + pidx -> row 64 of the rhs.
                # The qT rows are inserted by DMA to keep DVE/ACT off the
                # step-boundary critical path.
                mT = smp.tile([128, 128], f16, tag="mT")
                nc.sync.dma_start_transpose(mT, mcat)
                qasm = qsp.tile([65, 512], f16, tag="qasm")
                if CFG['qasm_engine'] == 'dma':
                    nc.sync.dma_start(out=qasm[0:64, :], in_=qT[h][:, lc * 512:(lc + 1) * 512])
                elif CFG['qasm_engine'] == 'dve':
                    nc.vector.tensor_copy(qasm[0:64, :], qT[h][:, lc * 512:(lc + 1) * 512])
                else:
                    nc.scalar.copy(qasm[0:64, :], qT[h][:, lc * 512:(lc + 1) * 512])
                nc.sync.dma_start(out=qasm[64:65, :], in_=mT[0:4, :])
                return qasm

            pools = {}

            def p2_mm(h, g, qasm, pts):
                if CFG.get('wide_p1'):
                    # narrow pass-2: one n-chunk per PSUM tile (1 bank)
                    t2 = pools['p2'].tile([128, 512], f32, tag="p2", name="t2")
                    nc.tensor.matmul(
                        t2, kT[h][:, g * 128:(g + 1) * 128],
                        qasm, start=True, stop=True)
                    pt = sbp.tile([128, 512], f16, tag="pt")
                    nc.scalar.activation(pt, t2, ACT_EXP)
                    pts[g] = pt
                    return
                # pass-2 [n, l] with -max folded in + exp for n-pair g
                t2 = pools['p2'].tile([128, 1024], f32, tag="p2", name="t2")
                for j in range(2):
                    n_ = g * 2 + j
                    nc.tensor.matmul(
                        t2[:, j * 512:(j + 1) * 512],
                        kT[h][:, n_ * 128:(n_ + 1) * 128],
                        qasm, start=True, stop=True)
                pt = sbp.tile([128, 1024], f16, tag="pt")
                nc.scalar.activation(pt, t2, ACT_EXP)
                pts[g] = pt

            def p2_av(h, g, pts, zt):
                if CFG.get('wide_p1'):
                    nc.tensor.matmul(
                        zt, v_all[:, g, h, :], pts.pop(g),
                        start=(g == 0), stop=(g == 15))
                    return
                # attn@V for n-pair g (one group behind exp to hide latency)
                for j in range(2):
                    n_ = g * 2 + j
                    nc.tensor.matmul(
                        zt, v_all[:, n_, h, :],
                        pts[g][:, j * 512:(j + 1) * 512],
                        start=(n_ == 0), stop=(n_ == 15))

            def p2_finish(h, lc, zt, last=False):
                if CFG.get('zu_batch'):
                    # stage into a per-head buffer; one DMA per head
                    if CFG['zsb_engine'] == 'act' and not last:
                        nc.scalar.copy(zout_sb[:, lc, :], zt)
                    else:
                        nc.vector.tensor_copy(zout_sb[:, lc, :], zt)
                    if lc == 3:
                        nc.sync.dma_start(out=zu_d[h, :, :], in_=zout_sb)
                    return
                zsb = sbp.tile([65, 512], f32, tag="zsb")
                if CFG['zsb_engine'] == 'act' and not last:
                    nc.scalar.copy(zsb, zt)
                else:
                    nc.vector.tensor_copy(zsb, zt)
                nc.sync.dma_start(out=zu_d[h, :, lc * 512:(lc + 1) * 512], in_=zsb)

            steps = [(h, lc) for h in range(HL) for lc in range(S // 512)]
            NS = len(steps)
            qasms = {}           # step index -> qasm tile
            p1_state = {}        # step index -> (mcat, red)
            red_slots = {}       # (h, lc, ls) -> [slots used]
            p1_done = 0          # count of fully-emitted pass-1 steps

            def emit_p1_step_chunk(i, g):
                # emit pass-1 chunk g of step i (allocating state at g==0),
                # finishing with the transpose/qasm assembly after g==7
                h, lc = steps[i]
                if g == 0:
                    p1_state[i] = (smp.tile([128, 128], f16, name="mcat", tag="mcat"),
                                   smp.tile([128, 16], f16, name="red", tag="red"))
                mcat, red = p1_state[i]
                p1_chunk(h, lc, g, mcat, red)
                if g == 7:
                    qasms[i] = p1_finish(h, lc, mcat, red)
                    del p1_state[i]

            # ---- phase A: projections + pass-1 of the first D steps ----
            with tc.tile_pool(name="psA", bufs=1, space="PSUM") as psAq, \
                 tc.tile_pool(name="psAv", bufs=CFG.get("psav_bufs", 2), space="PSUM") as psAv:

                def qk_unit(pack, sc):
                    h0, h1 = 2 * pack, 2 * pack + 1
                    ssl = slice(sc * 512, (sc + 1) * 512)
                    pq = psAq.tile([128, 512], f32, tag="pq")
                    for c in range(KC):
                        nc.tensor.matmul(
                            pq, wq_sb[:, c, pack * 128:(pack + 1) * 128],
                            xT_sb[:, c, ssl], start=(c == 0), stop=(c == KC - 1))
                    pk = psAq.tile([128, 512], f32, tag="pk")
                    for c in range(KC):
                        nc.tensor.matmul(
                            pk, wk_sb[:, c, pack * 128:(pack + 1) * 128],
                            xT_sb[:, c, ssl], start=(c == 0), stop=(c == KC - 1))
                    nc.scalar.copy(qT[h0][:, ssl], pq[0:64, :])
                    nc.scalar.copy(qT[h1][:, ssl], pq[64:128, :])
                    if CFG.get('k_copy_dve'):
                        nc.vector.tensor_copy(kT[h0][0:64, ssl], pk[0:64, :])
                        nc.vector.tensor_copy(kT[h1][0:64, ssl], pk[64:128, :])
                    else:
                        nc.scalar.copy(kT[h0][0:64, ssl], pk[0:64, :])
                        nc.scalar.copy(kT[h1][0:64, ssl], pk[64:128, :])

                def v_unit(n_):
                    pv = psAv.tile([128, HL * 64], f32, tag="pv")
                    for c in range(KC):
                        nc.tensor.matmul(
                            pv, xT_sb[:, c, n_ * 128:(n_ + 1) * 128],
                            wv_sb[:, c, :], start=(c == 0), stop=(c == KC - 1))
                    nc.vector.tensor_copy(
                        v_all[:, n_, :, 0:64],
                        pv.rearrange("p (h d) -> p h d", h=HL))

                for sc in range(S // 512):
                    qk_unit(0, sc)
                if CFG.get('v_first'):
                    units = ([("v", n_, None) for n_ in range(16)]
                             + [("qk", 1, sc) for sc in range(S // 512)]
                             + [("qk", 2, sc) for sc in range(S // 512)])
                else:
                    units = ([("qk", 1, sc) for sc in range(S // 512)]
                             + [("qk", 2, sc) for sc in range(S // 512)]
                             + [("v", n_, None) for n_ in range(16)])
                pre_chunks = [(i, g) for i in range(min(D, 8)) for g in range(8)]
                ci = 0
                skip = CFG.get('pre_skip', 6)
                for ui, u in enumerate(units):
                    if u[0] == "qk":
                        qk_unit(u[1], u[2])
                    else:
                        v_unit(u[1])
                    if ui < skip:
                        continue
                    want = (ui + 1 - skip) * len(pre_chunks) // max(1, len(units) - skip)
                    while ci < want:
                        i, g = pre_chunks[ci]
                        emit_p1_step_chunk(i, g)
                        ci += 1
                while ci < len(pre_chunks):
                    i, g = pre_chunks[ci]
                    emit_p1_step_chunk(i, g)
                    ci += 1
                p1_done = min(D, 8)

            # ---- phase B ----
            with tc.tile_pool(name="p2", bufs=2, space="PSUM") as p2p_, \
                 tc.tile_pool(name="ztp", bufs=(2 if CFG.get("wide_p1") else 1), space="PSUM") as ztp:
                pools['p2'] = p2p_
                TRAIL = 2 if CFG.get('wide_p1') else 1
                NG = 16 if CFG.get('wide_p1') else 8
                # remaining pass-1 chunks, optionally spread at a fractional
                # rate so the pipeline drain blends into the steady state
                queue = [(iq, gq) for iq in range(D, NS) for gq in range(8)]
                total_slots = NS * 8 - CFG.get('rate_slack', 0)
                ci = 0
                for i in range(NS):
                    h, lc = steps[i]
                    zt = ztp.tile([65, 512], f32, name="zt", tag="zt")
                    pts = {}
                    for g in range(NG + TRAIL):
                        if g < NG:
                            if g % (NG // 8) == 0:
                                gi = i * 8 + g // (NG // 8)
                                want = min(len(queue), (gi + 1) * len(queue) // total_slots
                                           if CFG.get('rate_slack', 0) else
                                           min(len(queue), (i < NS - D) * ((i * 8) + g // (NG // 8) + 1)))
                                if not CFG.get('rate_slack', 0):
                                    want = min(len(queue), gi + 1)
                                while ci < want:
                                    iq, gq = queue[ci]
                                    if CFG.get('p1_prio'):
                                        with tc.high_priority(offset=CFG['p1_prio']):
                                            emit_p1_step_chunk(iq, gq)
                                    else:
                                        emit_p1_step_chunk(iq, gq)
                                    ci += 1
                            p2_mm(h, g, qasms[i], pts)
                        if g >= TRAIL:
                            p2_av(h, g - TRAIL, pts, zt)
                    p2_finish(h, lc, zt, last=(i >= NS - 3))
                    del qasms[i]
                while ci < len(queue):
                    iq, gq = queue[ci]
                    emit_p1_step_chunk(iq, gq)
                    ci += 1
    nc.finalize()
    return nc


def _get_nc():
    if "nc" not in _CACHE:
        _CACHE["nc"] = build_bass()
    return _CACHE["nc"]


def _prep_in_maps(x, wq, wk, wv):
    in_maps = []
    for c in range(N_CORES):
        b, half = c // 2, c % 2
        hs = range(half * HL, (half + 1) * HL)
        xT = np.ascontiguousarray(x[b].T).astype(np.float16)
        wqc = (np.concatenate([wq[h] for h in hs], axis=1).astype(np.float32)
               * 0.125).astype(np.float16)
        wkc = np.concatenate([wk[h] for h in hs], axis=1).astype(np.float16)
        wvc = np.concatenate([wv[h] for h in hs], axis=1).astype(np.float16)
        if CFG.get('fused_in'):
            in_maps.append({"xw": np.ascontiguousarray(
                np.concatenate([xT, wqc, wkc, wvc], axis=1))})
        else:
            in_maps.append({"xT": xT, "wq": wqc, "wk": wkc, "wv": wvc})
    return in_maps


def _postprocess(results, wc):
    out = np.empty((B, S, 64), np.float32)
    wcT = np.ascontiguousarray(wc.T).astype(np.float32)
    for b in range(B):
        zparts = []
        for half in range(2):
            zu = results[b * 2 + half]["zu"]          # [HL, 65, S] f32
            z = zu[:, :64, :] / zu[:, 64:65, :]       # [HL, 64, S]
            zparts.append(z.transpose(2, 0, 1).reshape(S, HL * 64))
        out[b] = np.concatenate(zparts, axis=1) @ wcT
    return out


def kernel(x, wq, wk, wv, wc):
    from concourse.bass_utils import run_bass_kernel_spmd
    nc = _get_nc()
    in_maps = _prep_in_maps(np.asarray(x), np.asarray(wq), np.asarray(wk), np.asarray(wv))
    res = run_bass_kernel_spmd(nc, in_maps, list(range(N_CORES))).results
    return _postprocess(res, np.asarray(wc))


# revision 52
# speedup vs baseline: 1.0006x; 1.0006x over previous
"""Multi-head attention TRN2 kernel (8 NeuronCores).

Problem: B=4, S=2048, D_IN=768, H=12, D_HEAD=64.
  q/k/v = einsum('hkd,bsk->bhsd', w{q,k,v}, x)
  out   = einsum('ij,bsj->bsi', wc, softmax(q k^T / 8) v  concat-heads)

Sharding: 8 cores = (batch b in 0..3) x (head-half in 0..1), 6 heads per core.
Each core computes, per head: Q^T,K^T [64,2048] fp16 projections, a
max-finding scores pass in [l,n] layout (per-block DVE max-reduces
straight from PSUM; DVE is the only engine that can free-axis-reduce
and may read at most one PSUM operand per instruction), a second scores
pass in [n,l] layout with the per-row max folded in via an augmented
contraction row, exp on the scalar engine (PSUM fp32 -> SBUF fp16),
and attn@V with an appended
ones-column producing unnormalized z^T plus the softmax normalizer Z.
The host divides by Z, concatenates heads and applies the (tiny) output
projection in fp32 BLAS.

All matmuls run in fp16 (fp32 PSUM accumulation).
"""

import numpy as np

B, S, D_IN, H, D_HEAD = 4, 2048, 768, 12, 64
HL = H // 2          # heads per core
KC = D_IN // 128     # k chunks
N_CORES = 8

_CACHE = {}
CFG = {'shared_pool': False, 'zsb_engine': 'act', 'qasm_engine': 'dma', 'p1_prio': 30, 'act_blocks': frozenset(), 'combine_engine': 'dve', 'depth': 3, 'pre_skip': 11, 'v_first': True, 'sbp_bufs': 3, 'psav_bufs': 3, 'k_copy_dve': False, 'split_x0': False, 'wide_p1': False, 'rate_slack': 8, 'rot_p1': False, 'fused_in': True, 'fused_step': 1}


def build_bass():
    import concourse.bass as bass
    import concourse.bacc as bacc
    import concourse.mybir as mybir
    import concourse.tile as tile
    from contextlib import ExitStack

    f16 = mybir.dt.float16
    f32 = mybir.dt.float32
    AX = mybir.AxisListType
    ALU = mybir.AluOpType
    ACT_EXP = mybir.ActivationFunctionType.Exp

    nc = bacc.Bacc()
    XW = S + 3 * HL * 64
    if CFG.get('fused_in'):
        xw_d = nc.declare_dram_parameter("xw", [D_IN, XW], f16, isOutput=False)
    else:
        xT_d = nc.declare_dram_parameter("xT", [D_IN, S], f16, isOutput=False)
        wq_d = nc.declare_dram_parameter("wq", [D_IN, HL * 64], f16, isOutput=False)
        wk_d = nc.declare_dram_parameter("wk", [D_IN, HL * 64], f16, isOutput=False)
        wv_d = nc.declare_dram_parameter("wv", [D_IN, HL * 64], f16, isOutput=False)
    zu_d = nc.declare_dram_parameter("zu", [HL, 65, S], f32, isOutput=True)

    with tile.TileContext(nc) as tc, ExitStack() as ctx:
        consts = ctx.enter_context(tc.tile_pool(name="consts", bufs=1))

        # ---- persistent SBUF tensors ----
        if CFG.get('fused_in'):
            # all inputs in one tensor -> 6 chunk DMAs instead of 24,
            # collapsing the serialized HWDGE descriptor-gen chain
            xw_sb = consts.tile([128, KC, XW], f16)
            xT_sb = xw_sb[:, :, 0:S]
            wq_sb = xw_sb[:, :, S:S + HL * 64]
            wk_sb = xw_sb[:, :, S + HL * 64:S + 2 * HL * 64]
            wv_sb = xw_sb[:, :, S + 2 * HL * 64:S + 3 * HL * 64]
            step = CFG.get('fused_step', 1)
            if CFG.get('multi_q'):
                qs = [nc.sync, nc.scalar]
                for c0 in range(0, KC, step):
                    qs[(c0 // step) % len(qs)].dma_start(
                        out=xw_sb[:, c0:c0 + step, :],
                        in_=xw_d[c0 * 128:(c0 + step) * 128, :].rearrange(
                            "(c p) w -> p c w", c=step))
            else:
                for c0 in range(0, KC, step):
                    nc.sync.dma_start(
                        out=xw_sb[:, c0:c0 + step, :],
                        in_=xw_d[c0 * 128:(c0 + step) * 128, :].rearrange(
                            "(c p) w -> p c w", c=step))
        else:
            xT_sb = consts.tile([128, KC, S], f16)
            wq_sb = consts.tile([128, KC, HL * 64], f16)
            wk_sb = consts.tile([128, KC, HL * 64], f16)
            wv_sb = consts.tile([128, KC, HL * 64], f16)
        if (not CFG.get('fused_in')) and CFG.get('split_x0') == 'lead':
            # lead slice: wq c0 + first 512 cols of xT c0 land first so the
            # first projection matmul starts ~2us earlier
            nc.sync.dma_start(out=wq_sb[:, 0, :], in_=wq_d[0:128, :])
            nc.sync.dma_start(out=xT_sb[:, 0, 0:512], in_=xT_d[0:128, 0:512])
            nc.sync.dma_start(out=wk_sb[:, 0, :], in_=wk_d[0:128, :])
            nc.sync.dma_start(out=xT_sb[:, 0, 512:2048], in_=xT_d[0:128, 512:2048])
            for c in range(1, KC):
                nc.sync.dma_start(out=xT_sb[:, c, :], in_=xT_d[c * 128:(c + 1) * 128, :])
                nc.sync.dma_start(out=wq_sb[:, c, :], in_=wq_d[c * 128:(c + 1) * 128, :])
                nc.sync.dma_start(out=wk_sb[:, c, :], in_=wk_d[c * 128:(c + 1) * 128, :])
        elif (not CFG.get('fused_in')) and CFG.get('split_x0'):
            nc.sync.dma_start(out=wq_sb[:, 0, :], in_=wq_d[0:128, :])
            for sc in range(4):
                ssl = slice(sc * 512, (sc + 1) * 512)
                nc.sync.dma_start(out=xT_sb[:, 0, ssl], in_=xT_d[0:128, ssl])
            nc.sync.dma_start(out=wk_sb[:, 0, :], in_=wk_d[0:128, :])
            for c in range(1, KC):
                nc.sync.dma_start(out=xT_sb[:, c, :], in_=xT_d[c * 128:(c + 1) * 128, :])
                nc.sync.dma_start(out=wq_sb[:, c, :], in_=wq_d[c * 128:(c + 1) * 128, :])
                nc.sync.dma_start(out=wk_sb[:, c, :], in_=wk_d[c * 128:(c + 1) * 128, :])
        elif not CFG.get('fused_in'):
            for c in range(KC):
                nc.sync.dma_start(out=xT_sb[:, c, :], in_=xT_d[c * 128:(c + 1) * 128, :])
                nc.sync.dma_start(out=wq_sb[:, c, :], in_=wq_d[c * 128:(c + 1) * 128, :])
                nc.sync.dma_start(out=wk_sb[:, c, :], in_=wk_d[c * 128:(c + 1) * 128, :])
        if not CFG.get('fused_in'):
            for c in range(KC):
                nc.sync.dma_start(out=wv_sb[:, c, :], in_=wv_d[c * 128:(c + 1) * 128, :])

        if CFG.get('zu_batch'):
            zout_sb = consts.tile([65, 4, 512], f32, name="zout_sb")

        # per-head Q^T [64, S];  K~^T [65, S] with ones row;  V~ [128, 16, h, 65] with ones col
        qT = [consts.tile([64, S], f16, name=f"qT{h}", tag=f"qT{h}") for h in range(HL)]
        kT = [consts.tile([65, S], f16, name=f"kT{h}", tag=f"kT{h}") for h in range(HL)]
        v_all = consts.tile([128, 16, HL, 65], f16)
        for h in range(HL):
            nc.gpsimd.memset(kT[h][64:65, :], 1.0)
        nc.gpsimd.memset(v_all[:, :, :, 64:65], 1.0)


        # ---- attention, software-pipelined at depth D over phase A ----
        # steps = (head, l-chunk).  pass-1 (max-finding, DVE-bound) of step
        # i+1 is interleaved at per-group granularity with pass-2
        # (exp + attn@V) of step i.  Additionally the first D steps' pass-1
        # is emitted under the phase-A projection matmuls (PE-bound): pq/pk
        # are single-buffered so psA takes only 4 banks, and the 3-bank
        # pass-1 pool opens alongside it.  After psA closes, pass-2's 4
        # banks + zt open in the freed space.
        D = CFG['depth']
        from contextlib import ExitStack as _ES
        _pools = _ES()
        p1p = _pools.enter_context(tc.tile_pool(name="p1", bufs=(2 if CFG.get("wide_p1") else (1 if CFG.get('rot_p1') else 3)), space="PSUM"))
        if CFG.get('rot_p1'):
            t1r = p1p.tile([128, 1536], f32, name="t1r", tag="t1")
            rot = {'nb': 0}
        with _pools, tc.tile_pool(name="sbp", bufs=CFG.get("sbp_bufs", 3)) as sbp, \
             tc.tile_pool(name="qsp", bufs=D + 2) as qsp, \
             tc.tile_pool(name="smp", bufs=D + 1) as smp:

            def p1_chunk(h, lc, g, mcat, red):
                # pass-1 pair g (ls = g//2, n-half = g%2): two 512-n score
                # blocks, each max-reduced into a red column.  DVE can read
                # only one PSUM operand per instruction, so blocks reduce
                # independently; a few per step go via ACT copy + fp16
                # reduce to balance engine load.  After the last block of an
                # ls, the four red columns collapse (negated) into mcat[:,ls].
                ls, half = divmod(g, 2)
                l0 = lc * 512 + ls * 128
                if CFG.get('rot_p1'):
                    # manual 3-bank rotation: banks (nb, nb+1); non-wrapping
                    # pairs get one 1024-wide reduce, wrapping pairs two
                    # narrow ones.  WAR on region slices replaces pool frees.
                    nb = rot['nb']
                    b2 = (nb + 1) % 3
                    rot['nb'] = (nb + 2) % 3
                    for j, b in enumerate((nb, b2)):
                        nc.tensor.matmul(
                            t1r[:, b * 512:(b + 1) * 512], qT[h][:, l0:l0 + 128],
                            kT[h][0:64, (2 * half + j) * 512:(2 * half + j + 1) * 512],
                            start=True, stop=True)
                    sbase = 4 * ls
                    ns_used = red_slots.setdefault((h, lc, ls), [0])
                    if b2 == nb + 1:
                        nc.vector.tensor_reduce(
                            red[:, sbase + ns_used[0]:sbase + ns_used[0] + 1],
                            t1r[:, nb * 512:(nb + 2) * 512], axis=AX.X, op=ALU.max)
                        ns_used[0] += 1
                    else:
                        for b in (nb, b2):
                            nc.vector.tensor_reduce(
                                red[:, sbase + ns_used[0]:sbase + ns_used[0] + 1],
                                t1r[:, b * 512:(b + 1) * 512], axis=AX.X, op=ALU.max)
                            ns_used[0] += 1
                    if half == 1:
                        nc.vector.tensor_reduce(
                            mcat[:, ls:ls + 1], red[:, sbase:sbase + ns_used[0]],
                            axis=AX.X, op=ALU.max, negate=True)
                        del red_slots[(h, lc, ls)]
                    return
                if CFG.get('wide_p1'):
                    t1 = p1p.tile([128, 1024], f32, tag="t1", name="t1")
                    for j in range(2):
                        nc.tensor.matmul(
                            t1[:, j * 512:(j + 1) * 512], qT[h][:, l0:l0 + 128],
                            kT[h][0:64, (2 * half + j) * 512:(2 * half + j + 1) * 512],
                            start=True, stop=True)
                    nc.vector.tensor_reduce(
                        red[:, 2 * ls + half:2 * ls + half + 1],
                        t1, axis=AX.X, op=ALU.max)
                    if half == 1:
                        nc.vector.tensor_reduce(
                            mcat[:, ls:ls + 1], red[:, 2 * ls:2 * ls + 2],
                            axis=AX.X, op=ALU.max, negate=True)
                    return
                for j in range(2):
                    blk = g * 2 + j
                    tj = p1p.tile([128, 512], f32, tag="t1", name="tj")
                    nc.tensor.matmul(
                        tj, qT[h][:, l0:l0 + 128],
                        kT[h][0:64, (2 * half + j) * 512:(2 * half + j + 1) * 512],
                        start=True, stop=True)
                    if blk in CFG['act_blocks']:
                        cp = sbp.tile([128, 512], f16, tag="cp", name="cp")
                        with tc.high_priority(offset=CFG.get('cp_prio', 200)):
                            nc.scalar.copy(cp, tj)
                        nc.vector.tensor_reduce(
                            red[:, 4 * ls + 2 * half + j: 4 * ls + 2 * half + j + 1],
                            cp, axis=AX.X, op=ALU.max)
                    else:
                        nc.vector.tensor_reduce(
                            red[:, 4 * ls + 2 * half + j: 4 * ls + 2 * half + j + 1],
                            tj, axis=AX.X, op=ALU.max)
                if half == 1:
                    nc.vector.tensor_reduce(
                        mcat[:, ls:ls + 1], red[:, 4 * ls:4 * ls + 4],
                        axis=AX.X, op=ALU.max, negate=True)

            def p1_finish(h, lc, mcat, red):
                # -max cols [128, 0:4] -> XBAR DMA transpose -> mT[0:4, :]
                # holds -max for l = ls*128 + pidx -> row 64 of the rhs.
                # The qT rows are inserted by DMA to keep DVE/ACT off the
                # step-boundary critical path.
                mT = smp.tile([128, 128], f16, tag="mT")
                nc.sync.dma_start_transpose(mT, mcat)
                qasm = qsp.tile([65, 512], f16, tag="qasm")
                if CFG['qasm_engine'] == 'dma':
                    nc.sync.dma_start(out=qasm[0:64, :], in_=qT[h][:, lc * 512:(lc + 1) * 512])
                elif CFG['qasm_engine'] == 'dve':
                    nc.vector.tensor_copy(qasm[0:64, :], qT[h][:, lc * 512:(lc + 1) * 512])
                else:
                    nc.scalar.copy(qasm[0:64, :], qT[h][:, lc * 512:(lc + 1) * 512])
                nc.sync.dma_start(out=qasm[64:65, :], in_=mT[0:4, :])
                return qasm

            pools = {}

            def p2_mm(h, g, qasm, pts):
                if CFG.get('wide_p1'):
                    # narrow pass-2: one n-chunk per PSUM tile (1 bank)
                    t2 = pools['p2'].tile([128, 512], f32, tag="p2", name="t2")
                    nc.tensor.matmul(
                        t2, kT[h][:, g * 128:(g + 1) * 128],
                        qasm, start=True, stop=True)
                    pt = sbp.tile([128, 512], f16, tag="pt")
                    nc.scalar.activation(pt, t2, ACT_EXP)
                    pts[g] = pt
                    return
                # pass-2 [n, l] with -max folded in + exp for n-pair g
                t2 = pools['p2'].tile([128, 1024], f32, tag="p2", name="t2")
                for j in range(2):
                    n_ = g * 2 + j
                    nc.tensor.matmul(
                        t2[:, j * 512:(j + 1) * 512],
                        kT[h][:, n_ * 128:(n_ + 1) * 128],
                        qasm, start=True, stop=True)
                pt = sbp.tile([128, 1024], f16, tag="pt")
                nc.scalar.activation(pt, t2, ACT_EXP)
                pts[g] = pt

            def p2_av(h, g, pts, zt):
                if CFG.get('wide_p1'):
                    nc.tensor.matmul(
                        zt, v_all[:, g, h, :], pts.pop(g),
                        start=(g == 0), stop=(g == 15))
                    return
                # attn@V for n-pair g (one group behind exp to hide latency)
                for j in range(2):
                    n_ = g * 2 + j
                    nc.tensor.matmul(
                        zt, v_all[:, n_, h, :],
                        pts[g][:, j * 512:(j + 1) * 512],
                        start=(n_ == 0), stop=(n_ == 15))

            def p2_finish(h, lc, zt, last=False):
                if CFG.get('zu_batch'):
                    # stage into a per-head buffer; one DMA per head
                    if CFG['zsb_engine'] == 'act' and not last:
                        nc.scalar.copy(zout_sb[:, lc, :], zt)
                    else:
                        nc.vector.tensor_copy(zout_sb[:, lc, :], zt)
                    if lc == 3:
                        nc.sync.dma_start(out=zu_d[h, :, :], in_=zout_sb)
                    return
                zsb = sbp.tile([65, 512], f32, tag="zsb")
                if CFG['zsb_engine'] == 'act' and not last:
                    nc.scalar.copy(zsb, zt)
                else:
                    nc.vector.tensor_copy(zsb, zt)
                nc.sync.dma_start(out=zu_d[h, :, lc * 512:(lc + 1) * 512], in_=zsb)

            steps = [(h, lc) for h in range(HL) for lc in range(S // 512)]
            NS = len(steps)
            qasms = {}           # step index -> qasm tile
            p1_state = {}        # step index -> (mcat, red)
            red_slots = {}       # (h, lc, ls) -> [slots used]
            p1_done = 0          # count of fully-emitted pass-1 steps

            def emit_p1_step_chunk(i, g):
                # emit pass-1 chunk g of step i (allocating state at g==0),
                # finishing with the transpose/qasm assembly after g==7
                h, lc = steps[i]
                if g == 0:
                    p1_state[i] = (smp.tile([128, 128], f16, name="mcat", tag="mcat"),
                                   smp.tile([128, 16], f16, name="red", tag="red"))
                mcat, red = p1_state[i]
                p1_chunk(h, lc, g, mcat, red)
                if g == 7:
                    qasms[i] = p1_finish(h, lc, mcat, red)
                    del p1_state[i]

            # ---- phase A: projections + pass-1 of the first D steps ----
            with tc.tile_pool(name="psA", bufs=1, space="PSUM") as psAq, \
                 tc.tile_pool(name="psAv", bufs=CFG.get("psav_bufs", 2), space="PSUM") as psAv:

                def qk_unit(pack, sc):
                    h0, h1 = 2 * pack, 2 * pack + 1
                    ssl = slice(sc * 512, (sc + 1) * 512)
                    pq = psAq.tile([128, 512], f32, tag="pq")
                    for c in range(KC):
                        nc.tensor.matmul(
                            pq, wq_sb[:, c, pack * 128:(pack + 1) * 128],
                            xT_sb[:, c, ssl], start=(c == 0), stop=(c == KC - 1))
                    pk = psAq.tile([128, 512], f32, tag="pk")
                    for c in range(KC):
                        nc.tensor.matmul(
                            pk, wk_sb[:, c, pack * 128:(pack + 1) * 128],
                            xT_sb[:, c, ssl], start=(c == 0), stop=(c == KC - 1))
                    nc.scalar.copy(qT[h0][:, ssl], pq[0:64, :])
                    nc.scalar.copy(qT[h1][:, ssl], pq[64:128, :])
                    if CFG.get('k_copy_dve'):
                        nc.vector.tensor_copy(kT[h0][0:64, ssl], pk[0:64, :])
                        nc.vector.tensor_copy(kT[h1][0:64, ssl], pk[64:128, :])
                    else:
                        nc.scalar.copy(kT[h0][0:64, ssl], pk[0:64, :])
                        nc.scalar.copy(kT[h1][0:64, ssl], pk[64:128, :])

                def v_unit(n_):
                    pv = psAv.tile([128, HL * 64], f32, tag="pv")
                    for c in range(KC):
                        nc.tensor.matmul(
                            pv, xT_sb[:, c, n_ * 128:(n_ + 1) * 128],
                            wv_sb[:, c, :], start=(c == 0), stop=(c == KC - 1))
                    nc.vector.tensor_copy(
                        v_all[:, n_, :, 0:64],
                        pv.rearrange("p (h d) -> p h d", h=HL))

                for sc in range(S // 512):
                    qk_unit(0, sc)
                if CFG.get('v_first'):
                    units = ([("v", n_, None) for n_ in range(16)]
                             + [("qk", 1, sc) for sc in range(S // 512)]
                             + [("qk", 2, sc) for sc in range(S // 512)])
                else:
                    units = ([("qk", 1, sc) for sc in range(S // 512)]
                             + [("qk", 2, sc) for sc in range(S // 512)]
                             + [("v", n_, None) for n_ in range(16)])
                pre_chunks = [(i, g) for i in range(min(D, 8)) for g in range(8)]
                ci = 0
                skip = CFG.get('pre_skip', 6)
                for ui, u in enumerate(units):
                    if u[0] == "qk":
                        qk_unit(u[1], u[2])
                    else:
                        v_unit(u[1])
                    if ui < skip:
                        continue
                    want = (ui + 1 - skip) * len(pre_chunks) // max(1, len(units) - skip)
                    while ci < want:
                        i, g = pre_chunks[ci]
                        emit_p1_step_chunk(i, g)
                        ci += 1
                while ci < len(pre_chunks):
                    i, g = pre_chunks[ci]
                    emit_p1_step_chunk(i, g)
                    ci += 1
                p1_done = min(D, 8)

            # ---- phase B ----
            with tc.tile_pool(name="p2", bufs=2, space="PSUM") as p2p_, \
                 tc.tile_pool(name="ztp", bufs=(2 if CFG.get("wide_p1") else 1), space="PSUM") as ztp:
                pools['p2'] = p2p_
                TRAIL = 2 if CFG.get('wide_p1') else 1
                NG = 16 if CFG.get('wide_p1') else 8
                # remaining pass-1 chunks, optionally spread at a fractional
                # rate so the pipeline drain blends into the steady state
                queue = [(iq, gq) for iq in range(D, NS) for gq in range(8)]
                total_slots = NS * 8 - CFG.get('rate_slack', 0)
                ci = 0
                for i in range(NS):
                    h, lc = steps[i]
                    zt = ztp.tile([65, 512], f32, name="zt", tag="zt")
                    pts = {}
                    for g in range(NG + TRAIL):
                        if g < NG:
                            if g % (NG // 8) == 0:
                                gi = i * 8 + g // (NG // 8)
                                want = min(len(queue), (gi + 1) * len(queue) // total_slots
                                           if CFG.get('rate_slack', 0) else
                                           min(len(queue), (i < NS - D) * ((i * 8) + g // (NG // 8) + 1)))
                                if not CFG.get('rate_slack', 0):
                                    want = min(len(queue), gi + 1)
                                while ci < want:
                                    iq, gq = queue[ci]
                                    if CFG.get('p1_prio'):
                                        with tc.high_priority(offset=CFG['p1_prio']):
                                            emit_p1_step_chunk(iq, gq)
                                    else:
                                        emit_p1_step_chunk(iq, gq)
                                    ci += 1
                            p2_mm(h, g, qasms[i], pts)
                        if g >= TRAIL:
                            p2_av(h, g - TRAIL, pts, zt)
                    p2_finish(h, lc, zt, last=(i >= NS - 3))
                    del qasms[i]
                while ci < len(queue):
                    iq, gq = queue[ci]
                    emit_p1_step_chunk(iq, gq)
                    ci += 1
    nc.finalize()
    return nc


def _get_nc():
    if "nc" not in _CACHE:
        _CACHE["nc"] = build_bass()
    return _CACHE["nc"]


def _prep_in_maps(x, wq, wk, wv):
    in_maps = []
    for c in range(N_CORES):
        b, half = c // 2, c % 2
        hs = range(half * HL, (half + 1) * HL)
        xT = np.ascontiguousarray(x[b].T).astype(np.float16)
        wqc = (np.concatenate([wq[h] for h in hs], axis=1).astype(np.float32)
               * 0.125).astype(np.float16)
        wkc = np.concatenate([wk[h] for h in hs], axis=1).astype(np.float16)
        wvc = np.concatenate([wv[h] for h in hs], axis=1).astype(np.float16)
        if CFG.get('fused_in'):
            in_maps.append({"xw": np.ascontiguousarray(
                np.concatenate([xT, wqc, wkc, wvc], axis=1))})
        else:
            in_maps.append({"xT": xT, "wq": wqc, "wk": wkc, "wv": wvc})
    return in_maps


def _postprocess(results, wc):
    out = np.empty((B, S, 64), np.float32)
    wcT = np.ascontiguousarray(wc.T).astype(np.float32)
    for b in range(B):
        zparts = []
        for half in range(2):
            zu = results[b * 2 + half]["zu"]          # [HL, 65, S] f32
            z = zu[:, :64, :] / zu[:, 64:65, :]       # [HL, 64, S]
            zparts.append(z.transpose(2, 0, 1).reshape(S, HL * 64))
        out[b] = np.concatenate(zparts, axis=1) @ wcT
    return out


def kernel(x, wq, wk, wv, wc):
    from concourse.bass_utils import run_bass_kernel_spmd
    nc = _get_nc()
    in_maps = _prep_in_maps(np.asarray(x), np.asarray(wq), np.asarray(wk), np.asarray(wv))
    res = run_bass_kernel_spmd(nc, in_maps, list(range(N_CORES))).results
    return _postprocess(res, np.asarray(wc))


# revision 54
# speedup vs baseline: 1.0008x; 1.0002x over previous
"""Multi-head attention TRN2 kernel (8 NeuronCores).

Problem: B=4, S=2048, D_IN=768, H=12, D_HEAD=64.
  q/k/v = einsum('hkd,bsk->bhsd', w{q,k,v}, x)
  out   = einsum('ij,bsj->bsi', wc, softmax(q k^T / 8) v  concat-heads)

Sharding: 8 cores = (batch b in 0..3) x (head-half in 0..1), 6 heads per core.
Each core computes, per head: Q^T,K^T [64,2048] fp16 projections, a
max-finding scores pass in [l,n] layout (per-block DVE max-reduces
straight from PSUM; DVE is the only engine that can free-axis-reduce
and may read at most one PSUM operand per instruction), a second scores
pass in [n,l] layout with the per-row max folded in via an augmented
contraction row, exp on the scalar engine (PSUM fp32 -> SBUF fp16),
and attn@V with an appended
ones-column producing unnormalized z^T plus the softmax normalizer Z.
The host divides by Z, concatenates heads and applies the (tiny) output
projection in fp32 BLAS.

All matmuls run in fp16 (fp32 PSUM accumulation).
"""

import numpy as np

B, S, D_IN, H, D_HEAD = 4, 2048, 768, 12, 64
HL = H // 2          # heads per core
KC = D_IN // 128     # k chunks
N_CORES = 8

_CACHE = {}
CFG = {'shared_pool': False, 'zsb_engine': 'act', 'qasm_engine': 'dma', 'p1_prio': 30, 'act_blocks': frozenset(), 'combine_engine': 'dve', 'depth': 3, 'pre_skip': 11, 'v_first': True, 'sbp_bufs': 3, 'psav_bufs': 3, 'k_copy_dve': False, 'split_x0': False, 'wide_p1': False, 'rate_slack': 8, 'rot_p1': False, 'fused_in': True, 'fused_step': 1, 'c0_split': True}


def build_bass():
    import concourse.bass as bass
    import concourse.bacc as bacc
    import concourse.mybir as mybir
    import concourse.tile as tile
    from contextlib import ExitStack

    f16 = mybir.dt.float16
    f32 = mybir.dt.float32
    AX = mybir.AxisListType
    ALU = mybir.AluOpType
    ACT_EXP = mybir.ActivationFunctionType.Exp

    nc = bacc.Bacc()
    XW = S + 3 * HL * 64
    if CFG.get('fused_in'):
        xw_d = nc.declare_dram_parameter("xw", [D_IN, XW], f16, isOutput=False)
    else:
        xT_d = nc.declare_dram_parameter("xT", [D_IN, S], f16, isOutput=False)
        wq_d = nc.declare_dram_parameter("wq", [D_IN, HL * 64], f16, isOutput=False)
        wk_d = nc.declare_dram_parameter("wk", [D_IN, HL * 64], f16, isOutput=False)
        wv_d = nc.declare_dram_parameter("wv", [D_IN, HL * 64], f16, isOutput=False)
    zu_d = nc.declare_dram_parameter("zu", [HL, 65, S], f32, isOutput=True)

    with tile.TileContext(nc) as tc, ExitStack() as ctx:
        consts = ctx.enter_context(tc.tile_pool(name="consts", bufs=1))

        # ---- persistent SBUF tensors ----
        if CFG.get('fused_in'):
            # all inputs in one tensor -> 6 chunk DMAs instead of 24,
            # collapsing the serialized HWDGE descriptor-gen chain
            xw_sb = consts.tile([128, KC, XW], f16)
            xT_sb = xw_sb[:, :, 0:S]
            wq_sb = xw_sb[:, :, S:S + HL * 64]
            wk_sb = xw_sb[:, :, S + HL * 64:S + 2 * HL * 64]
            wv_sb = xw_sb[:, :, S + 2 * HL * 64:S + 3 * HL * 64]
            step = CFG.get('fused_step', 1)
            if CFG.get('c0_split'):
                # first chunk split: small weights-part lands first so the
                # first projection matmul starts earlier
                nc.sync.dma_start(out=xw_sb[:, 0, S:], in_=xw_d[0:128, S:])
                nc.sync.dma_start(out=xw_sb[:, 0, 0:S], in_=xw_d[0:128, 0:S])
                for c0 in range(1, KC):
                    nc.sync.dma_start(out=xw_sb[:, c0, :], in_=xw_d[c0 * 128:(c0 + 1) * 128, :])
            elif CFG.get('multi_q'):
                qs = [nc.sync, nc.scalar]
                for c0 in range(0, KC, step):
                    qs[(c0 // step) % len(qs)].dma_start(
                        out=xw_sb[:, c0:c0 + step, :],
                        in_=xw_d[c0 * 128:(c0 + step) * 128, :].rearrange(
                            "(c p) w -> p c w", c=step))
            else:
                for c0 in range(0, KC, step):
                    nc.sync.dma_start(
                        out=xw_sb[:, c0:c0 + step, :],
                        in_=xw_d[c0 * 128:(c0 + step) * 128, :].rearrange(
                            "(c p) w -> p c w", c=step))
        else:
            xT_sb = consts.tile([128, KC, S], f16)
            wq_sb = consts.tile([128, KC, HL * 64], f16)
            wk_sb = consts.tile([128, KC, HL * 64], f16)
            wv_sb = consts.tile([128, KC, HL * 64], f16)
        if (not CFG.get('fused_in')) and CFG.get('split_x0') == 'lead':
            # lead slice: wq c0 + first 512 cols of xT c0 land first so the
            # first projection matmul starts ~2us earlier
            nc.sync.dma_start(out=wq_sb[:, 0, :], in_=wq_d[0:128, :])
            nc.sync.dma_start(out=xT_sb[:, 0, 0:512], in_=xT_d[0:128, 0:512])
            nc.sync.dma_start(out=wk_sb[:, 0, :], in_=wk_d[0:128, :])
            nc.sync.dma_start(out=xT_sb[:, 0, 512:2048], in_=xT_d[0:128, 512:2048])
            for c in range(1, KC):
                nc.sync.dma_start(out=xT_sb[:, c, :], in_=xT_d[c * 128:(c + 1) * 128, :])
                nc.sync.dma_start(out=wq_sb[:, c, :], in_=wq_d[c * 128:(c + 1) * 128, :])
                nc.sync.dma_start(out=wk_sb[:, c, :], in_=wk_d[c * 128:(c + 1) * 128, :])
        elif (not CFG.get('fused_in')) and CFG.get('split_x0'):
            nc.sync.dma_start(out=wq_sb[:, 0, :], in_=wq_d[0:128, :])
            for sc in range(4):
                ssl = slice(sc * 512, (sc + 1) * 512)
                nc.sync.dma_start(out=xT_sb[:, 0, ssl], in_=xT_d[0:128, ssl])
            nc.sync.dma_start(out=wk_sb[:, 0, :], in_=wk_d[0:128, :])
            for c in range(1, KC):
                nc.sync.dma_start(out=xT_sb[:, c, :], in_=xT_d[c * 128:(c + 1) * 128, :])
                nc.sync.dma_start(out=wq_sb[:, c, :], in_=wq_d[c * 128:(c + 1) * 128, :])
                nc.sync.dma_start(out=wk_sb[:, c, :], in_=wk_d[c * 128:(c + 1) * 128, :])
        elif not CFG.get('fused_in'):
            for c in range(KC):
                nc.sync.dma_start(out=xT_sb[:, c, :], in_=xT_d[c * 128:(c + 1) * 128, :])
                nc.sync.dma_start(out=wq_sb[:, c, :], in_=wq_d[c * 128:(c + 1) * 128, :])
                nc.sync.dma_start(out=wk_sb[:, c, :], in_=wk_d[c * 128:(c + 1) * 128, :])
        if not CFG.get('fused_in'):
            for c in range(KC):
                nc.sync.dma_start(out=wv_sb[:, c, :], in_=wv_d[c * 128:(c + 1) * 128, :])

        if CFG.get('zu_batch'):
            zout_sb = consts.tile([65, 4, 512], f32, name="zout_sb")

        # per-head Q^T [64, S];  K~^T [65, S] with ones row;  V~ [128, 16, h, 65] with ones col
        qT = [consts.tile([64, S], f16, name=f"qT{h}", tag=f"qT{h}") for h in range(HL)]
        kT = [consts.tile([65, S], f16, name=f"kT{h}", tag=f"kT{h}") for h in range(HL)]
        v_all = consts.tile([128, 16, HL, 65], f16)
        for h in range(HL):
            nc.gpsimd.memset(kT[h][64:65, :], 1.0)
        nc.gpsimd.memset(v_all[:, :, :, 64:65], 1.0)


        # ---- attention, software-pipelined at depth D over phase A ----
        # steps = (head, l-chunk).  pass-1 (max-finding, DVE-bound) of step
        # i+1 is interleaved at per-group granularity with pass-2
        # (exp + attn@V) of step i.  Additionally the first D steps' pass-1
        # is emitted under the phase-A projection matmuls (PE-bound): pq/pk
        # are single-buffered so psA takes only 4 banks, and the 3-bank
        # pass-1 pool opens alongside it.  After psA closes, pass-2's 4
        # banks + zt open in the freed space.
        D = CFG['depth']
        from contextlib import ExitStack as _ES
        _pools = _ES()
        p1p = _pools.enter_context(tc.tile_pool(name="p1", bufs=(2 if CFG.get("wide_p1") else (1 if CFG.get('rot_p1') else 3)), space="PSUM"))
        if CFG.get('rot_p1'):
            t1r = p1p.tile([128, 1536], f32, name="t1r", tag="t1")
            rot = {'nb': 0}
        with _pools, tc.tile_pool(name="sbp", bufs=CFG.get("sbp_bufs", 3)) as sbp, \
             tc.tile_pool(name="qsp", bufs=D + 2) as qsp, \
             tc.tile_pool(name="smp", bufs=D + 1) as smp:

            def p1_chunk(h, lc, g, mcat, red):
                # pass-1 pair g (ls = g//2, n-half = g%2): two 512-n score
                # blocks, each max-reduced into a red column.  DVE can read
                # only one PSUM operand per instruction, so blocks reduce
                # independently; a few per step go via ACT copy + fp16
                # reduce to balance engine load.  After the last block of an
                # ls, the four red columns collapse (negated) into mcat[:,ls].
                ls, half = divmod(g, 2)
                l0 = lc * 512 + ls * 128
                if CFG.get('rot_p1'):
                    # manual 3-bank rotation: banks (nb, nb+1); non-wrapping
                    # pairs get one 1024-wide reduce, wrapping pairs two
                    # narrow ones.  WAR on region slices replaces pool frees.
                    nb = rot['nb']
                    b2 = (nb + 1) % 3
                    rot['nb'] = (nb + 2) % 3
                    for j, b in enumerate((nb, b2)):
                        nc.tensor.matmul(
                            t1r[:, b * 512:(b + 1) * 512], qT[h][:, l0:l0 + 128],
                            kT[h][0:64, (2 * half + j) * 512:(2 * half + j + 1) * 512],
                            start=True, stop=True)
                    sbase = 4 * ls
                    ns_used = red_slots.setdefault((h, lc, ls), [0])
                    if b2 == nb + 1:
                        nc.vector.tensor_reduce(
                            red[:, sbase + ns_used[0]:sbase + ns_used[0] + 1],
                            t1r[:, nb * 512:(nb + 2) * 512], axis=AX.X, op=ALU.max)
                        ns_used[0] += 1
                    else:
                        for b in (nb, b2):
                            nc.vector.tensor_reduce(
                                red[:, sbase + ns_used[0]:sbase + ns_used[0] + 1],
                                t1r[:, b * 512:(b + 1) * 512], axis=AX.X, op=ALU.max)
                            ns_used[0] += 1
                    if half == 1:
                        nc.vector.tensor_reduce(
                            mcat[:, ls:ls + 1], red[:, sbase:sbase + ns_used[0]],
                            axis=AX.X, op=ALU.max, negate=True)
                        del red_slots[(h, lc, ls)]
                    return
                if CFG.get('wide_p1'):
                    t1 = p1p.tile([128, 1024], f32, tag="t1", name="t1")
                    for j in range(2):
                        nc.tensor.matmul(
                            t1[:, j * 512:(j + 1) * 512], qT[h][:, l0:l0 + 128],
                            kT[h][0:64, (2 * half + j) * 512:(2 * half + j + 1) * 512],
                            start=True, stop=True)
                    nc.vector.tensor_reduce(
                        red[:, 2 * ls + half:2 * ls + half + 1],
                        t1, axis=AX.X, op=ALU.max)
                    if half == 1:
                        nc.vector.tensor_reduce(
                            mcat[:, ls:ls + 1], red[:, 2 * ls:2 * ls + 2],
                            axis=AX.X, op=ALU.max, negate=True)
                    return
                for j in range(2):
                    blk = g * 2 + j
                    tj = p1p.tile([128, 512], f32, tag="t1", name="tj")
                    nc.tensor.matmul(
                        tj, qT[h][:, l0:l0 + 128],
                        kT[h][0:64, (2 * half + j) * 512:(2 * half + j + 1) * 512],
                        start=True, stop=True)
                    if blk in CFG['act_blocks']:
                        cp = sbp.tile([128, 512], f16, tag="cp", name="cp")
                        with tc.high_priority(offset=CFG.get('cp_prio', 200)):
                            nc.scalar.copy(cp, tj)
                        nc.vector.tensor_reduce(
                            red[:, 4 * ls + 2 * half + j: 4 * ls + 2 * half + j + 1],
                            cp, axis=AX.X, op=ALU.max)
                    else:
                        nc.vector.tensor_reduce(
                            red[:, 4 * ls + 2 * half + j: 4 * ls + 2 * half + j + 1],
                            tj, axis=AX.X, op=ALU.max)
                if half == 1:
                    nc.vector.tensor_reduce(
                        mcat[:, ls:ls + 1], red[:, 4 * ls:4 * ls + 4],
                        axis=AX.X, op=ALU.max, negate=True)

            def p1_finish(h, lc, mcat, red):
                # -max cols [128, 0:4] -> XBAR DMA transpose -> mT[0:4, :]
                # holds -max for l = ls*128 + pidx -> row 64 of the rhs.
                # The qT rows are inserted by DMA to keep DVE/ACT off the
                # step-boundary critical path.
                mT = smp.tile([128, 128], f16, tag="mT")
                nc.sync.dma_start_transpose(mT, mcat)
                qasm = qsp.tile([65, 512], f16, tag="qasm")
                if CFG['qasm_engine'] == 'dma':
                    nc.sync.dma_start(out=qasm[0:64, :], in_=qT[h][:, lc * 512:(lc + 1) * 512])
                elif CFG['qasm_engine'] == 'dve':
                    nc.vector.tensor_copy(qasm[0:64, :], qT[h][:, lc * 512:(lc + 1) * 512])
                else:
                    nc.scalar.copy(qasm[0:64, :], qT[h][:, lc * 512:(lc + 1) * 512])
                nc.sync.dma_start(out=qasm[64:65, :], in_=mT[0:4, :])
                return qasm

            pools = {}

            def p2_mm(h, g, qasm, pts):
                if CFG.get('wide_p1'):
                    # narrow pass-2: one n-chunk per PSUM tile (1 bank)
                    t2 = pools['p2'].tile([128, 512], f32, tag="p2", name="t2")
                    nc.tensor.matmul(
                        t2, kT[h][:, g * 128:(g + 1) * 128],
                        qasm, start=True, stop=True)
                    pt = sbp.tile([128, 512], f16, tag="pt")
                    nc.scalar.activation(pt, t2, ACT_EXP)
                    pts[g] = pt
                    return
                # pass-2 [n, l] with -max folded in + exp for n-pair g
                t2 = pools['p2'].tile([128, 1024], f32, tag="p2", name="t2")
                for j in range(2):
                    n_ = g * 2 + j
                    nc.tensor.matmul(
                        t2[:, j * 512:(j + 1) * 512],
                        kT[h][:, n_ * 128:(n_ + 1) * 128],
                        qasm, start=True, stop=True)
                pt = sbp.tile([128, 1024], f16, tag="pt")
                nc.scalar.activation(pt, t2, ACT_EXP)
                pts[g] = pt

            def p2_av(h, g, pts, zt):
                if CFG.get('wide_p1'):
                    nc.tensor.matmul(
                        zt, v_all[:, g, h, :], pts.pop(g),
                        start=(g == 0), stop=(g == 15))
                    return
                # attn@V for n-pair g (one group behind exp to hide latency)
                for j in range(2):
                    n_ = g * 2 + j
                    nc.tensor.matmul(
                        zt, v_all[:, n_, h, :],
                        pts[g][:, j * 512:(j + 1) * 512],
                        start=(n_ == 0), stop=(n_ == 15))

            def p2_finish(h, lc, zt, last=False):
                if CFG.get('zu_batch'):
                    # stage into a per-head buffer; one DMA per head
                    if CFG['zsb_engine'] == 'act' and not last:
                        nc.scalar.copy(zout_sb[:, lc, :], zt)
                    else:
                        nc.vector.tensor_copy(zout_sb[:, lc, :], zt)
                    if lc == 3:
                        nc.sync.dma_start(out=zu_d[h, :, :], in_=zout_sb)
                    return
                zsb = sbp.tile([65, 512], f32, tag="zsb")
                if CFG['zsb_engine'] == 'act' and not last:
                    nc.scalar.copy(zsb, zt)
                else:
                    nc.vector.tensor_copy(zsb, zt)
                nc.sync.dma_start(out=zu_d[h, :, lc * 512:(lc + 1) * 512], in_=zsb)

            steps = [(h, lc) for h in range(HL) for lc in range(S // 512)]
            NS = len(steps)
            qasms = {}           # step index -> qasm tile
            p1_state = {}        # step index -> (mcat, red)
            red_slots = {}       # (h, lc, ls) -> [slots used]
            p1_done = 0          # count of fully-emitted pass-1 steps

            def emit_p1_step_chunk(i, g):
                # emit pass-1 chunk g of step i (allocating state at g==0),
                # finishing with the transpose/qasm assembly after g==7
                h, lc = steps[i]
                if g == 0:
                    p1_state[i] = (smp.tile([128, 128], f16, name="mcat", tag="mcat"),
                                   smp.tile([128, 16], f16, name="red", tag="red"))
                mcat, red = p1_state[i]
                p1_chunk(h, lc, g, mcat, red)
                if g == 7:
                    qasms[i] = p1_finish(h, lc, mcat, red)
                    del p1_state[i]

            # ---- phase A: projections + pass-1 of the first D steps ----
            with tc.tile_pool(name="psA", bufs=1, space="PSUM") as psAq, \
                 tc.tile_pool(name="psAv", bufs=CFG.get("psav_bufs", 2), space="PSUM") as psAv:

                def qk_unit(pack, sc):
                    h0, h1 = 2 * pack, 2 * pack + 1
                    ssl = slice(sc * 512, (sc + 1) * 512)
                    pq = psAq.tile([128, 512], f32, tag="pq")
                    for c in range(KC):
                        nc.tensor.matmul(
                            pq, wq_sb[:, c, pack * 128:(pack + 1) * 128],
                            xT_sb[:, c, ssl], start=(c == 0), stop=(c == KC - 1))
                    pk = psAq.tile([128, 512], f32, tag="pk")
                    for c in range(KC):
                        nc.tensor.matmul(
                            pk, wk_sb[:, c, pack * 128:(pack + 1) * 128],
                            xT_sb[:, c, ssl], start=(c == 0), stop=(c == KC - 1))
                    nc.scalar.copy(qT[h0][:, ssl], pq[0:64, :])
                    nc.scalar.copy(qT[h1][:, ssl], pq[64:128, :])
                    if CFG.get('k_copy_dve'):
                        nc.vector.tensor_copy(kT[h0][0:64, ssl], pk[0:64, :])
                        nc.vector.tensor_copy(kT[h1][0:64, ssl], pk[64:128, :])
                    else:
                        nc.scalar.copy(kT[h0][0:64, ssl], pk[0:64, :])
                        nc.scalar.copy(kT[h1][0:64, ssl], pk[64:128, :])

                def v_unit(n_):
                    pv = psAv.tile([128, HL * 64], f32, tag="pv")
                    for c in range(KC):
                        nc.tensor.matmul(
                            pv, xT_sb[:, c, n_ * 128:(n_ + 1) * 128],
                            wv_sb[:, c, :], start=(c == 0), stop=(c == KC - 1))
                    nc.vector.tensor_copy(
                        v_all[:, n_, :, 0:64],
                        pv.rearrange("p (h d) -> p h d", h=HL))

                for sc in range(S // 512):
                    qk_unit(0, sc)
                if CFG.get('v_first'):
                    units = ([("v", n_, None) for n_ in range(16)]
                             + [("qk", 1, sc) for sc in range(S // 512)]
                             + [("qk", 2, sc) for sc in range(S // 512)])
                else:
                    units = ([("qk", 1, sc) for sc in range(S // 512)]
                             + [("qk", 2, sc) for sc in range(S // 512)]
                             + [("v", n_, None) for n_ in range(16)])
                pre_chunks = [(i, g) for i in range(min(D, 8)) for g in range(8)]
                ci = 0
                skip = CFG.get('pre_skip', 6)
                for ui, u in enumerate(units):
                    if u[0] == "qk":
                        qk_unit(u[1], u[2])
                    else:
                        v_unit(u[1])
                    if ui < skip:
                        continue
                    want = (ui + 1 - skip) * len(pre_chunks) // max(1, len(units) - skip)
                    while ci < want:
                        i, g = pre_chunks[ci]
                        emit_p1_step_chunk(i, g)
                        ci += 1
                while ci < len(pre_chunks):
                    i, g = pre_chunks[ci]
                    emit_p1_step_chunk(i, g)
                    ci += 1
                p1_done = min(D, 8)

            # ---- phase B ----
            with tc.tile_pool(name="p2", bufs=2, space="PSUM") as p2p_, \
                 tc.tile_pool(name="ztp", bufs=(2 if CFG.get("wide_p1") else 1), space="PSUM") as ztp:
                pools['p2'] = p2p_
                TRAIL = 2 if CFG.get('wide_p1') else 1
                NG = 16 if CFG.get('wide_p1') else 8
                # remaining pass-1 chunks, optionally spread at a fractional
                # rate so the pipeline drain blends into the steady state
                queue = [(iq, gq) for iq in range(D, NS) for gq in range(8)]
                total_slots = NS * 8 - CFG.get('rate_slack', 0)
                ci = 0
                for i in range(NS):
                    h, lc = steps[i]
                    zt = ztp.tile([65, 512], f32, name="zt", tag="zt")
                    pts = {}
                    for g in range(NG + TRAIL):
                        if g < NG:
                            if g % (NG // 8) == 0:
                                gi = i * 8 + g // (NG // 8)
                                want = min(len(queue), (gi + 1) * len(queue) // total_slots
                                           if CFG.get('rate_slack', 0) else
                                           min(len(queue), (i < NS - D) * ((i * 8) + g // (NG // 8) + 1)))
                                if not CFG.get('rate_slack', 0):
                                    want = min(len(queue), gi + 1)
                                while ci < want:
                                    iq, gq = queue[ci]
                                    if CFG.get('p1_prio'):
                                        with tc.high_priority(offset=CFG['p1_prio']):
                                            emit_p1_step_chunk(iq, gq)
                                    else:
                                        emit_p1_step_chunk(iq, gq)
                                    ci += 1
                            p2_mm(h, g, qasms[i], pts)
                        if g >= TRAIL:
                            p2_av(h, g - TRAIL, pts, zt)
                    p2_finish(h, lc, zt, last=(i >= NS - 3))
                    del qasms[i]
                while ci < len(queue):
                    iq, gq = queue[ci]
                    emit_p1_step_chunk(iq, gq)
                    ci += 1
    nc.finalize()
    return nc


def _get_nc():
    if "nc" not in _CACHE:
        _CACHE["nc"] = build_bass()
    return _CACHE["nc"]


def _prep_in_maps(x, wq, wk, wv):
    in_maps = []
    for c in range(N_CORES):
        b, half = c // 2, c % 2
        hs = range(half * HL, (half + 1) * HL)
        xT = np.ascontiguousarray(x[b].T).astype(np.float16)
        wqc = (np.concatenate([wq[h] for h in hs], axis=1).astype(np.float32)
               * 0.125).astype(np.float16)
        wkc = np.concatenate([wk[h] for h in hs], axis=1).astype(np.float16)
        wvc = np.concatenate([wv[h] for h in hs], axis=1).astype(np.float16)
        if CFG.get('fused_in'):
            in_maps.append({"xw": np.ascontiguousarray(
                np.concatenate([xT, wqc, wkc, wvc], axis=1))})
        else:
            in_maps.append({"xT": xT, "wq": wqc, "wk": wkc, "wv": wvc})
    return in_maps


def _postprocess(results, wc):
    out = np.empty((B, S, 64), np.float32)
    wcT = np.ascontiguousarray(wc.T).astype(np.float32)
    for b in range(B):
        zparts = []
        for half in range(2):
            zu = results[b * 2 + half]["zu"]          # [HL, 65, S] f32
            z = zu[:, :64, :] / zu[:, 64:65, :]       # [HL, 64, S]
            zparts.append(z.transpose(2, 0, 1).reshape(S, HL * 64))
        out[b] = np.concatenate(zparts, axis=1) @ wcT
    return out


def kernel(x, wq, wk, wv, wc):
    from concourse.bass_utils import run_bass_kernel_spmd
    nc = _get_nc()
    in_maps = _prep_in_maps(np.asarray(x), np.asarray(wq), np.asarray(wk), np.asarray(wv))
    res = run_bass_kernel_spmd(nc, in_maps, list(range(N_CORES))).results
    return _postprocess(res, np.asarray(wc))
